# revision 28
# baseline (speedup 1.0000x reference)
"""Trainium2 Bass kernel for nn_DIYloss_1709396984424.

Loss: for binary labels, mean over (one, zero) pairs of (1 + p[l] - p[k])^2
where p = sigmoid(pred_Y). With q = 1 - p, each pair term is
(q_k + p_l)^2, so the L^2 sum has the closed form

    pair_sum = n0*alpha + 2*beta*gamma + n1*delta
      n1 = sum(m),        n0 = L - n1        (m = one-mask)
      s1 = sum(m*p),      s2 = sum(m*p^2)
      gamma = sum((1-m)*p),  delta = sum((1-m)*p^2)
      alpha = sum(m*q^2) = n1 - 2*s1 + s2,  beta = sum(m*q) = n1 - s1

    loss = pair_sum / max(n1*n0, 1)

Each of the 8 cores receives the full (replicated) input and computes the
full scalar on-device; core 0's output is returned. The two inputs are
packed host-side into one [128,128] f32 buffer (int32 labels bitcast into
the second half) so a single DMA brings everything in.

Schedule (per core): the SP DMA trigger is hoisted in front of the
framework preamble barrier so the ~640ns preamble (const-AP memsets +
all-engine barrier) hides entirely under the ~2.4us DMA latency. ACT does
the sigmoid; DVE produces the five masked row-sum columns with fused
accum_out; one tiny PE matmul reduces the partition axis; the epilogue is
9 small DVE ops in 4 semaphore hops (the 3-term pair_sum is one fused
multiply+row-accum over a 3-lane vector reading totals from PSUM via
pointer scalars). The 4-byte result goes out via SP register store (no
DMA). No reset tail: each execution's waits are satisfied monotonically,
and re-execution with the same NEFF state stays deterministic (verified
by the double-call check in test.py).
"""

import numpy as np

try:
    import concourse.bass as bass  # noqa: F401
except ImportError:  # pragma: no cover - grading env should have it on path
    import sys

    sys.path.insert(0, "/opt/trn_rl_repo")
    import concourse.bass as bass  # noqa: F401

from concourse import bacc, mybir
from concourse.bass_utils import run_bass_kernel_spmd

L = 8192
P = 128
F = L // P  # 64
N_CORES = 8

_f32 = mybir.dt.float32
_i32 = mybir.dt.int32
_Alu = mybir.AluOpType
_Act = mybir.ActivationFunctionType

_built = None


def _build_v4(tail="none"):
    """Depth-optimized schedule.

    Stats phase is 2 sem-hops deep after p: DVE does mp = m*p (s1) then
    s2 = mp*p, while ACT's sigmoid and Square carry Tp and Tp2 as fused
    row-sum accumulators. The totals row is (s1, s2, n1, Tp, Tp2); the
    epilogue rebuilds V = (alpha, gamma, delta) = (n1,Tp,Tp2) + s2*(1,0,-1)
    + s1*(-2,-1,0) and u = (z, 2*beta, n1) in two fused pointer-scalar hops
    each, reading the totals straight from PSUM (the scalar-pointer operand
    is exempt from the one-PSUM-operand rule), then pair = sum(u*V) with a
    fused row-accum, reciprocal, multiply — 4 hops from totals to loss.
    """
    nc = bacc.Bacc(
        "TRN2", debug=False, target_bir_lowering=False, num_devices=N_CORES
    )
    xin_d = nc.dram_tensor("xin", [P, 2 * F], _f32, kind="ExternalInput")
    out_d = nc.dram_tensor("out", [1, 1], _f32, kind="ExternalOutput")

    with (
        nc.sbuf_tensor("xt", [P, 2 * F], _f32) as xt,
        nc.sbuf_tensor("p", [P, F], _f32) as p,
        nc.sbuf_tensor("m", [P, F], _f32) as m,
        nc.sbuf_tensor("mp", [P, F], _f32) as mp,
        nc.sbuf_tensor("p2", [P, F], _f32) as p2,
        nc.sbuf_tensor("sc1", [P, F], _f32) as sc1,
        nc.sbuf_tensor("stats", [P, 8], _f32) as stats,
        nc.sbuf_tensor("ones", [P, 1], _f32) as ones,
        nc.sbuf_tensor("w", [1, 32], _f32) as w,
        nc.psum_tensor("acc", [1, 8], _f32) as acc,
        nc.semaphore("s_in") as s_in,
        nc.semaphore("s_act") as s_act,
        nc.semaphore("s_dve") as s_dve,
        nc.semaphore("s_pe") as s_pe,
    ):
        pred_v = xt[:, 0:F]
        true_v = xt[:, F : 2 * F].bitcast(_i32)

        def c(i, j=None):
            return w[0:1, i : (i + 1 if j is None else j)]

        # w cells: 0=1.0 | 1:4=cB=(1,0,-1) | 4:7=cL=(L,0,0) | 7:10=cU2=(-1,2,1)
        # 10:13=cA=(-2,-1,0) | 13:16=cU1=(0,-2,0)
        # 16:19=va->V | 19:22=ua->u | 22=z | 23=den | 24=pair | 25=rec
        # 26=loss | 27:30=pair product scratch

        dma_inst = nc.sync.dma_start(xt[:], xin_d[:]).then_inc(s_in, 16)

        ndve = 0

        def dv(inst, after=0):
            nonlocal ndve
            ndve += 1
            if after:
                inst._wait_ge(s_dve, after)
            inst.then_inc(s_dve, 1)
            return ndve

        i_zero = dv(nc.vector.memset(c(0, 16), 0.0))
        for cell, val in [
            (0, 1.0), (1, 1.0), (3, -1.0), (4, float(L)), (7, -1.0),
            (8, 2.0), (9, 1.0), (10, -2.0), (11, -1.0), (14, -2.0),
        ]:
            dv(nc.vector.memset(c(cell), val), after=i_zero)
        dv(nc.vector.memset(ones[:], 1.0))

        # stats cols: 0=s1, 1=s2, 2=n1, 3=Tp, 4=Tp2
        # int32 -> f32 cast + row-sum (the HW tensor-scalar reduce rejects
        # int inputs, so this stays a copy + reduce; both run before p lands)
        i_m = dv(nc.vector.tensor_copy(m[:], true_v)._wait_ge(s_in, 16))
        dv(
            nc.vector.tensor_reduce(
                stats[:, 2:3], m[:], axis=mybir.AxisListType.X, op=_Alu.add
            ),
            after=i_m,
        )
        # mp = float(true)*p computed straight from the int32 labels so the
        # only wait is s_act (the DMA edge is transitive through sigmoid's
        # s_in wait) — no sequencer park serializing the engine start
        i_mp = dv(
            nc.vector.scalar_tensor_tensor(
                out=mp[:], in0=true_v, scalar=1.0, in1=p[:],
                op0=_Alu.mult, op1=_Alu.mult, accum_out=stats[:, 0:1],
            )._wait_ge(s_act, 1)
        )
        dv(
            nc.vector.tensor_reduce(
                stats[:, 3:4], p[:], axis=mybir.AxisListType.X, op=_Alu.add
            )._wait_ge(s_act, 1)
        )  # Tp; fills the DVE engine slot while s2 waits on mp's update
        # depth 2: s2 = sum((m*p) * p)
        dv(
            nc.vector.scalar_tensor_tensor(
                out=sc1[:], in0=mp[:], scalar=1.0, in1=p[:],
                op0=_Alu.mult, op1=_Alu.mult, accum_out=stats[:, 1:2],
            ),
            after=i_mp,
        )
        # bridge: fold ACT's Square completion (s_act=2) into the s_dve count
        # so the matmul's single wait covers every stats producer; keeping all
        # s_dve updates on one engine keeps the count deterministic
        dv(nc.vector.wait_ge(s_act, 2))
        n_stats = ndve

        # --- epilogue: 4 sem-hops from totals to loss ---
        # hop 1 (all gated on s_pe only)
        i_va = dv(
            nc.vector.scalar_tensor_tensor(
                out=c(16, 19), in0=c(1, 4), scalar=acc[0:1, 1:2],
                in1=acc[0:1, 2:5], op0=_Alu.mult, op1=_Alu.add,
            )._wait_ge(s_pe, 1)
        )  # va = s2*(1,0,-1) + (n1,Tp,Tp2)
        i_ua = dv(
            nc.vector.scalar_tensor_tensor(
                out=c(19, 22), in0=c(13, 16), scalar=acc[0:1, 0:1],
                in1=c(4, 7), op0=_Alu.mult, op1=_Alu.add,
            )._wait_ge(s_pe, 1)
        )  # ua = s1*(0,-2,0) + (L,0,0)
        i_z = dv(
            nc.vector.scalar_tensor_tensor(
                out=c(22), in0=acc[0:1, 2:3], scalar=-1.0, in1=c(4),
                op0=_Alu.mult, op1=_Alu.add,
            )._wait_ge(s_pe, 1)
        )  # z = L - n1
        # hop 2
        i_V = dv(
            nc.vector.scalar_tensor_tensor(
                out=c(16, 19), in0=c(10, 13), scalar=acc[0:1, 0:1],
                in1=c(16, 19), op0=_Alu.mult, op1=_Alu.add,
            ),
            after=i_va,
        )  # V = s1*(-2,-1,0) + va = (alpha, gamma, delta)
        i_u = dv(
            nc.vector.scalar_tensor_tensor(
                out=c(19, 22), in0=c(7, 10), scalar=acc[0:1, 2:3],
                in1=c(19, 22), op0=_Alu.mult, op1=_Alu.add,
            ),
            after=i_ua,
        )  # u = n1*(-1,2,1) + ua = (z, 2*beta, n1)
        i_den = dv(
            nc.vector.scalar_tensor_tensor(
                out=c(23), in0=c(22), scalar=acc[0:1, 2:3], in1=c(0),
                op0=_Alu.mult, op1=_Alu.max,
            ),
            after=i_z,
        )  # den = max(z*n1, 1)
        # hop 3
        i_pair = dv(
            nc.vector.scalar_tensor_tensor(
                out=c(27, 30), in0=c(19, 22), scalar=1.0, in1=c(16, 19),
                op0=_Alu.mult, op1=_Alu.mult, accum_out=c(24),
            ),
            after=i_u,
        )  # pair = sum(u * V)
        i_rec = dv(nc.vector.reciprocal(c(25), c(23)), after=i_den)
        # hop 4
        dv(nc.vector.tensor_mul(c(26), c(24), c(25)), after=i_rec)
        n_all = ndve

        # plain sigmoid (no accum) so the Square can start 187ns earlier;
        # Tp is a DVE reduce instead
        nc.scalar.activation(p[:], pred_v, _Act.Sigmoid)._wait_ge(
            s_in, 16
        ).then_inc(s_act, 1)
        # Tp2 = sum(p^2) on ACT (Square shares the sigmoid table set). Its
        # completion is signaled on s_dve: the matmul's single wait below
        # covers both producers because DVE epilogue increments cannot occur
        # until s_pe fires, so s_dve == n_stats + 1 iff all DVE stats ops AND
        # this op are done.
        # the ACT engine is exclusive and issues in order, so the Square only
        # needs the input-data edge; it runs right after the sigmoid's engine
        # slot instead of waiting for sigmoid's (post-drain) semaphore
        nc.scalar.activation(
            p2[:], p[:], _Act.Square, accum_out=stats[:, 4:5]
        )._wait_ge(s_in, 16).then_inc(s_act, 1)

        nc.tensor.matmul(
            acc[0:1, 0:5], ones[:], stats[:, 0:5], start=True, stop=True
        )._wait_ge(s_dve, n_stats).then_inc(s_pe, 1)

        reg = nc.sync.alloc_register()
        nc.sync.reg_load(reg, c(26).bitcast(_i32))._wait_ge(s_dve, n_all)
        nc.sync.store(out_d[0:1, 0:1].bitcast(_i32), reg)
        assert tail == "none"

    entry = nc.main_func.blocks[0]
    raw = dma_inst.ins
    insts = entry.instructions
    insts.remove(raw)
    sp_drain = next(
        i
        for i, inst in enumerate(insts)
        if isinstance(inst, mybir.InstDrain) and inst.engine == mybir.EngineType.SP
    )
    insts.insert(sp_drain, raw)

    nc.compile()
    return nc


_build = _build_v4


def _pack(pred_Y, true_Y):
    xin = np.empty((P, 2 * F), dtype=np.float32)
    xin[:, 0:F] = np.ascontiguousarray(pred_Y, dtype=np.float32).reshape(P, F)
    xin[:, F : 2 * F] = (
        np.ascontiguousarray(true_Y, dtype=np.int32).reshape(P, F).view(np.float32)
    )
    return xin


def _run(pred_Y, true_Y, **hw_kwargs):
    global _built
    if _built is None:
        _built = _build()
    in_map = {"xin": _pack(pred_Y, true_Y)}
    res = run_bass_kernel_spmd(
        _built, [in_map] * N_CORES, list(range(N_CORES)), **hw_kwargs
    )
    out = np.asarray(res.results[0]["out"], dtype=np.float32).reshape(())
    return out, res


def kernel(pred_Y, true_Y):
    out, _ = _run(pred_Y, true_Y)
    return out
